# revision 25
# baseline (speedup 1.0000x reference)
"""Trainium2 Bass kernel for nn_CFI_Module (non-local attention block), fp8.

Per batch b (c=256, h=w=64 -> S=4096 spatial, viewed N=2048):
  phi   = W_phi   @ A   viewed (256, 2048);  theta = W_theta @ B likewise
  g     = W_g     @ [A;B] viewed (256, 2048)
  scores[n, m] = sum_cc theta_v[cc, n] phi_v[cc, m]
  attn = softmax over n (per column m);  y = attn^T-apply;  out = W_mask @ y + W_AB @ [A;B]

Sharding: 8 cores = 4 batches x 2-way split of the m (softmax-free) dim.
Softmax over n is core-local.  Host adds the two per-batch om partials; the
W_AB skip conv is per-strip (disjoint) per core.

Numerics (validated vs fp32 reference in numpy: l2 ~ 3.4e-3):
  - all attention-path matmuls in fp8 with DoubleRow perf mode (2 K-subtiles
    per instruction, 0.5 PE cycles per output element)
  - viewed channel p = 2*ch + hh (conv channel ch, spatial half hh), so the
    256-deep contractions split naturally into hh subtile pairs for DoubleRow
  - weights pre-scaled by 2^6 on host (e4m3-friendly), descaled via the exp
    activation scale (2^-12) and drain-time constant rescales
  - E = exp(scores) stored e5m2 unnormalized (max e^10.9 < e5m2 max)
  - GT = g^T * 2^14/Z stored e4m3 (fits normal range); y accumulates in f32
    PSUM, drained * 2^-7 to e4m3; om output = om * 2^9 in e4m3 (host unscale)
  - W_AB skip conv in fp8 hi/lo decomposition: W@x ~= Wh@xh + (Wh/16)@xl'
    + (Wl/16)@xh with xl' = 16*(x - fp8(x)); output f16 (dominant term,
    ~0.5% error, well within the 2e-2 gate)
"""
import sys

for _p in ("/opt/trn_rl_repo", "/root/.axon_site/_ro/trn_rl_repo"):
    if _p not in sys.path:
        sys.path.append(_p)

import numpy as np
from contextlib import ExitStack

import ml_dtypes
import concourse.bacc as bacc
import concourse.tile as tile
from concourse import mybir
from concourse.bass_utils import run_bass_kernel_spmd

F32 = mybir.dt.float32
F16 = mybir.dt.float16
E4 = mybir.dt.float8e4
E5 = mybir.dt.float8e5
E4NP = ml_dtypes.float8_e4m3
E5NP = ml_dtypes.float8_e5m2
F16NP = np.float16
DR = mybir.MatmulPerfMode.DoubleRow
EXP = mybir.ActivationFunctionType.Exp
MUL = mybir.AluOpType.mult

_NC_CACHE = {}

# resident y blocks (st, nb) accumulated during the exp chain (PSUM-limited)
RESIDENT = [(0, 0), (0, 1)]


def build_nc():
    nc = bacc.Bacc(target_bir_lowering=False, trn_type="TRN2")

    # ---- DRAM I/O ----
    B8_d = nc.dram_tensor("B8", [128, 2, 2, 2048], E4, kind="ExternalInput")
    AH8_d = nc.dram_tensor("AH8", [128, 2, 2048], E4, kind="ExternalInput")
    BH8_d = nc.dram_tensor("BH8", [128, 2, 2048], E4, kind="ExternalInput")
    ALO_d = nc.dram_tensor("ALO", [128, 2, 2048], E4, kind="ExternalInput")
    BLO_d = nc.dram_tensor("BLO", [128, 2, 2048], E4, kind="ExternalInput")
    # wth(2) wph(2) wgA(2) wgB(2) wmk(2)
    WQ1_d = nc.dram_tensor("WQ1", [128, 10, 128], E4, kind="ExternalInput")
    # wab: term(3) x och(2) x jstep(2) -> idx = term*4 + och*2 + jstep
    WQ2_d = nc.dram_tensor("WQ2", [128, 12, 2, 128], E4, kind="ExternalInput")
    OM_d = nc.dram_tensor("OM8", [2, 128, 4096], E4, kind="ExternalOutput")
    OW_d = nc.dram_tensor("OW", [2, 128, 2048], F16, kind="ExternalOutput")

    with tile.TileContext(nc) as tc:
        with ExitStack() as ctx:
            io = ctx.enter_context(tc.tile_pool(name="io", bufs=1))
            acts = ctx.enter_context(tc.tile_pool(name="acts", bufs=1))
            spool = ctx.enter_context(tc.tile_pool(name="spool", bufs=8))
            stg = ctx.enter_context(tc.tile_pool(name="stg", bufs=4))
            wstg = ctx.enter_context(tc.tile_pool(name="wstg", bufs=2))
            psS = ctx.enter_context(tc.tile_pool(name="psS", bufs=2, space="PSUM"))
            psY = ctx.enter_context(tc.tile_pool(name="psY", bufs=2, space="PSUM"))
            psG = ctx.enter_context(tc.tile_pool(name="psG", bufs=1, space="PSUM"))
            psW = ctx.enter_context(tc.tile_pool(name="psW", bufs=1, space="PSUM"))

            # ---- SBUF ----
            b8 = io.tile([128, 2, 2, 2048], E4, name="b8")
            ah8 = io.tile([128, 2, 2048], E4, name="ah8")
            bh8 = io.tile([128, 2, 2048], E4, name="bh8")
            alo = io.tile([128, 2, 2048], E4, name="alo")
            blo = io.tile([128, 2, 2048], E4, name="blo")
            wq1 = io.tile([128, 10, 128], E4, name="wq1")
            wq2 = io.tile([128, 12, 2, 128], E4, name="wq2")
            TH = acts.tile([128, 2, 2048], E4, name="TH")
            PH = acts.tile([128, 2, 1024], E4, name="PH")
            Et = acts.tile([128, 8, 2048], E5, name="Et")
            GT = acts.tile([128, 2, 8, 128], E4, name="GT")
            Y8 = acts.tile([128, 2, 2048], E4, name="Y8")

            wth = wq1[:, 0:2, :]
            wph = wq1[:, 2:4, :]
            wgA = wq1[:, 4:6, :]
            wgB = wq1[:, 6:8, :]

            # ---- input DMAs, ordered by first use ----
            nc.sync.dma_start(out=b8[:, :, 1, 0:1024], in_=B8_d[:, :, 1, 0:1024])
            nc.sync.dma_start(out=wq1, in_=WQ1_d[:, :, :])
            nc.sync.dma_start(out=b8[:, :, 1, 1024:2048],
                              in_=B8_d[:, :, 1, 1024:2048])
            # phi chunk 0 needs strip cols [0:128] and [1024:1152]
            nc.sync.dma_start(out=ah8[:, :, 0:128], in_=AH8_d[:, :, 0:128])
            nc.sync.dma_start(out=ah8[:, :, 1024:1152],
                              in_=AH8_d[:, :, 1024:1152])
            nc.sync.dma_start(out=ah8[:, :, 128:640], in_=AH8_d[:, :, 128:640])
            nc.sync.dma_start(out=ah8[:, :, 1152:1664],
                              in_=AH8_d[:, :, 1152:1664])
            nc.sync.dma_start(out=b8[:, :, 0, :], in_=B8_d[:, :, 0, :])
            nc.sync.dma_start(out=ah8[:, :, 640:1024], in_=AH8_d[:, :, 640:1024])
            nc.sync.dma_start(out=ah8[:, :, 1664:2048],
                              in_=AH8_d[:, :, 1664:2048])
            nc.sync.dma_start(out=bh8, in_=BH8_d[:, :, :])
            nc.sync.dma_start(out=wq2, in_=WQ2_d[:, :, :, :])
            nc.sync.dma_start(out=alo, in_=ALO_d[:, :, :])
            nc.sync.dma_start(out=blo, in_=BLO_d[:, :, :])

            # ---- theta conv: TH[ch, hh, n] over b8 quarters ----
            def theta_q(q, drain="v", pool="s"):
                qp, off = q % 2, 1024 * (q // 2)
                hh, nh = divmod(q, 2)
                dst = TH[:, hh, 1024 * nh:1024 * (nh + 1)]
                if pool == "s":
                    tp = psS.tile([128, 1024], F32, tag="s", name="tp")
                    for jj in range(2):
                        nc.tensor.matmul(
                            tp[:, 512 * jj:512 * (jj + 1)],
                            wth,
                            b8[:, :, qp, off + 512 * jj:off + 512 * (jj + 1)],
                            start=True, stop=True, perf_mode=DR,
                        )
                    if drain == "a":
                        nc.scalar.copy(dst, tp)
                    else:
                        nc.vector.tensor_copy(dst[:, 0:512], tp[:, 0:512])
                        nc.vector.tensor_copy(dst[:, 512:1024], tp[:, 512:1024])
                else:
                    for jj in range(2):
                        tp = psY.tile([128, 512], F32, tag="acc", name="tp")
                        nc.tensor.matmul(
                            tp, wth,
                            b8[:, :, qp, off + 512 * jj:off + 512 * (jj + 1)],
                            start=True, stop=True, perf_mode=DR,
                        )
                        nc.vector.tensor_copy(
                            dst[:, 512 * jj:512 * (jj + 1)], tp)

            theta_q(1)
            theta_q(3, "a")

            # ---- phi conv: PH[ch, hh, m] over strip-col ranges ----
            def phi_range(t0, t1, engine):
                pp = psY.tile([128, 512], F32, tag="acc", name="pp")
                nc.tensor.matmul(pp[:, 0:t1 - t0], wph, ah8[:, :, t0:t1],
                                 start=True, stop=True, perf_mode=DR)
                hh, m0 = divmod(t0, 1024)
                dst = PH[:, hh, m0:m0 + (t1 - t0)]
                if engine == "a":
                    nc.scalar.copy(dst, pp[:, 0:t1 - t0])
                else:
                    nc.vector.tensor_copy(dst, pp[:, 0:t1 - t0])

            phi_range(0, 128, "v")
            phi_range(1024, 1152, "a")
            phi_range(128, 640, "v")
            phi_range(1152, 1664, "v")

            # ---- ow (W_AB skip conv, fp8 hi/lo) job pieces ----
            # cg indexes 512-wide strip-col groups; och the out-channel half.
            ow_stages = {}

            def ow_job(och, cg):
                fw = psW.tile([128, 512], F32, tag="w", name="fw")
                c0, c1 = 512 * cg, 512 * (cg + 1)
                movers = {
                    0: (ah8[:, :, c0:c1], bh8[:, :, c0:c1]),
                    1: (alo[:, :, c0:c1], blo[:, :, c0:c1]),
                    2: (ah8[:, :, c0:c1], bh8[:, :, c0:c1]),
                }
                first = True
                with tc.high_priority(offset=-400000):
                    for term in range(3):
                        for jstep in range(2):
                            nc.tensor.matmul(
                                fw,
                                wq2[:, term * 4 + och * 2 + jstep, :, :],
                                movers[term][jstep],
                                start=first,
                                stop=(term == 2 and jstep == 1),
                                perf_mode=DR,
                            )
                            first = False
                key = (och, cg // 2)
                if key not in ow_stages:
                    ow_stages[key] = wstg.tile([128, 1024], F16, tag="wst",
                                               name=f"ow{och}_{cg // 2}")
                st_t = ow_stages[key]
                nc.vector.tensor_scalar_mul(
                    st_t[:, 512 * (cg % 2):512 * (cg % 2 + 1)], fw, 2.0 ** -6)
                if cg % 2 == 1:
                    nc.sync.dma_start(
                        out=OW_d[och, :, 1024 * (cg // 2):1024 * (cg // 2 + 1)],
                        in_=st_t,
                    )

            ow_jobs = [(och, cg) for cg in range(4) for och in range(2)]

            # ---- scores + exp chain, with GT convs and in-loop y ----
            zs = []

            def scores_exp(k, h2):
                sp = psS.tile([128, 1024], F32, tag="s", name="sp")
                for jj in range(2):
                    nc.tensor.matmul(
                        sp[:, 512 * jj:512 * (jj + 1)],
                        PH[:, :, 128 * k:128 * (k + 1)],
                        TH[:, :, 1024 * h2 + 512 * jj:1024 * h2 + 512 * (jj + 1)],
                        start=True, stop=True, perf_mode=DR,
                    )
                nc.scalar.activation(
                    out=Et[:, k, 1024 * h2:1024 * (h2 + 1)],
                    in_=sp,
                    func=EXP,
                    scale=2.0 ** -12,
                    accum_out=zs[k][:, h2:h2 + 1],
                )

            def gt_k(k, hot=False):
                # Z recip (fused *256 into the drain)
                from contextlib import nullcontext
                hp = tc.high_priority() if hot else nullcontext()
                with hp:
                    nc.vector.tensor_add(zs[k][:, 2:3], zs[k][:, 0:1],
                                         zs[k][:, 1:2])
                    nc.vector.reciprocal(zs[k][:, 3:4], zs[k][:, 2:3])
                gp = psG.tile([128, 512], F32, tag="g", name="gp")
                for st in range(2):
                    with tc.high_priority(offset=-500000):
                        nc.tensor.matmul(
                            gp[:, 128 * st:128 * (st + 1)],
                            ah8[:, :, 1024 * st + 128 * k:1024 * st + 128 * (k + 1)],
                            wgA,
                            start=True, stop=False, perf_mode=DR,
                        )
                        nc.tensor.matmul(
                            gp[:, 128 * st:128 * (st + 1)],
                            bh8[:, :, 1024 * st + 128 * k:
                                1024 * st + 128 * (k + 1)],
                            wgB,
                            start=False, stop=True, perf_mode=DR,
                        )
                hp2 = tc.high_priority() if hot else nullcontext()
                with hp2:
                    nc.vector.tensor_scalar(
                        GT[:, :, k, :], gp[:, 0:256],
                        zs[k][:, 3:4], 256.0, op0=MUL, op1=MUL,
                    )

            y_ps = {}
            y_pool_rr = [psY, psY]

            def y_step(st, nb, p):
                if (st, nb) not in y_ps:
                    pool = y_pool_rr.pop(0) if y_pool_rr else psY
                    y_ps[(st, nb)] = pool.tile([128, 512], F32,
                                               tag="acc" if pool is psY else
                                               ("w" if pool is psW else "g"),
                                               name=f"y{st}{nb}")
                with tc.high_priority(offset=-1000000):
                    nc.tensor.matmul(
                        y_ps[(st, nb)],
                        GT[:, st, 2 * p:2 * p + 2, :],
                        Et[:, 2 * p:2 * p + 2, 512 * nb:512 * (nb + 1)],
                        start=(p == 0), stop=(p == 3),
                        perf_mode=DR,
                    )

            # chain order: two n-half sweeps.  Sweep 1 (h2=0) is gated only
            # by B quarters q0/q2 + phi; sweep 2 (h2=1) by q1/q3 whose theta
            # convs/drains overlap sweep 1.  GT (needs Z = both halves) and
            # in-loop y ride sweep 2.
            for k in range(8):
                zs.append(spool.tile([128, 4], F32, tag="z", name=f"z{k}"))
            ow_i = 0
            for k in range(8):
                scores_exp(k, 1)
                if k == 3:
                    theta_q(0, pool="y")
                    theta_q(2, pool="y")
                    phi_range(640, 1024, "v")
                    phi_range(1664, 2048, "v")
                if k >= 4:
                    for _ in range(2):
                        if ow_i < len(ow_jobs):
                            ow_job(*ow_jobs[ow_i])
                            ow_i += 1
            for k in range(8):
                scores_exp(k, 0)
                gt_k(k, hot=(k >= 6))
                if k % 2 == 1 and k < 7:
                    for (st, nb) in RESIDENT:
                        y_step(st, nb, (k - 1) // 2)

            # ---- tail: finish y, drain Y8, mask conv, om out ----
            def y_drain(st, nb, engine):
                t = y_ps.pop((st, nb))
                dst = Y8[:, st, 512 * nb:512 * (nb + 1)]
                with tc.high_priority():
                    if engine == "v":
                        nc.vector.tensor_scalar_mul(dst, t, 2.0 ** -7)
                    else:
                        nc.scalar.mul(dst, t, 2.0 ** -7)

            om_rr = []
            om_eng = [0]
            om_stage = {}

            def om_single(st, nb, och):
                # psum: rotate over donated pools / psS halves
                kind, pool = om_rr[0]
                om_rr.append(om_rr.pop(0))
                if kind == "half":
                    base, half = pool
                    f = base[:, 512 * half:512 * (half + 1)]
                else:
                    f = pool.tile([128, 512], F32,
                                  tag="acc" if pool is psY else
                                  ("w" if pool is psW else "g"),
                                  name="f_om")
                nc.tensor.matmul(
                    f, wq1[:, 8 + och, :],
                    Y8[:, st, 512 * nb:512 * (nb + 1)],
                    start=True, stop=True,
                )
                key = (st, och)
                if key not in om_stage:
                    om_stage[key] = [stg.tile([128, 2048], E4, tag="om",
                                              name="s_om"), 0]
                ent = om_stage[key]
                sg = ent[0][:, 512 * nb:512 * (nb + 1)]
                # 10:6 ACT:DVE split (ACT is otherwise idle in the tail)
                if om_eng[0] % 8 in (0, 1, 3, 4, 6):
                    nc.scalar.copy(sg, f)
                else:
                    nc.vector.tensor_copy(sg, f)
                om_eng[0] += 1
                ent[1] += 1
                if ent[1] == 4:
                    nc.sync.dma_start(
                        out=OM_d[och, :, 2048 * st:2048 * (st + 1)],
                        in_=om_stage.pop(key)[0],
                    )

            # tail: psW and psG are free now -> widen the y rotation
            y_pool_rr.extend([psW, psG])
            for (st, nb) in RESIDENT:
                y_step(st, nb, 3)
            y_drain(*RESIDENT[0], "a")
            y_drain(*RESIDENT[1], "v")
            # om psum rotation: two psS halves + donated pools as they free up
            fS1 = psS.tile([128, 1024], F32, tag="s", name="fS1")
            fS2 = psS.tile([128, 1024], F32, tag="s", name="fS2")
            om_rr.extend([("half", (fS1, 0)), ("half", (fS1, 1)),
                          ("half", (fS2, 0)), ("half", (fS2, 1))])
            for och in range(2):
                om_single(0, 0, och)
                om_single(0, 1, och)
            rest = [(st, nb) for st in range(2) for nb in range(4)
                    if (st, nb) not in RESIDENT]
            ei = 0
            for (st, nb) in rest:
                for p in range(4):
                    y_step(st, nb, p)
                y_drain(st, nb, "v" if ei % 2 == 0 else "a")
                ei += 1
                for och in range(2):
                    om_single(st, nb, och)

    nc.compile()
    return nc


def _get_nc():
    if "nc" not in _NC_CACHE:
        _NC_CACHE["nc"] = build_nc()
    return _NC_CACHE["nc"]


def _prep_inputs(A, B, W_phi, W_theta, W_g, W_AB, W_mask):
    A = np.ascontiguousarray(np.asarray(A, np.float32)).reshape(4, 256, 4096)
    B = np.ascontiguousarray(np.asarray(B, np.float32)).reshape(4, 256, 4096)
    W_phi = np.asarray(W_phi, np.float32)
    W_theta = np.asarray(W_theta, np.float32)
    W_g = np.asarray(W_g, np.float32)
    W_AB = np.asarray(W_AB, np.float32)
    W_mask = np.asarray(W_mask, np.float32)

    def q8(x):
        return np.clip(x, -240.0, 240.0).astype(E4NP)

    A8 = q8(A)
    B8 = q8(B)
    Alo = q8((A - A8.astype(np.float32)) * 16.0)
    Blo = q8((B - B8.astype(np.float32)) * 16.0)

    def chansplit(w):
        # (ch_out, 256) -> [128 icl, 2 ich, ch_out]
        t = w.T.reshape(2, 128, -1)  # (ich, icl, ch)
        return t.transpose(1, 0, 2)

    wq1 = np.zeros((128, 10, 128), np.float32)
    wq1[:, 0:2, :] = chansplit(W_theta * 64.0)
    wq1[:, 2:4, :] = chansplit(W_phi * 64.0)
    wg = (W_g * 64.0).T.reshape(4, 128, 128).transpose(1, 0, 2)  # [jl, jc, chg]
    wq1[:, 4:6, :] = wg[:, 0:2, :]
    wq1[:, 6:8, :] = wg[:, 2:4, :]
    wmkT = (W_mask * 4.0).T  # (128 i, 256 oc)
    wq1[:, 8, :] = wmkT[:, 0:128]
    wq1[:, 9, :] = wmkT[:, 128:256]
    wq1 = q8(wq1)

    Whi = q8(W_AB * 64.0)
    Wlo = q8((W_AB * 64.0 - Whi.astype(np.float32)) * 16.0)
    terms = [Whi.astype(np.float32), Whi.astype(np.float32) / 16.0,
             Wlo.astype(np.float32) / 16.0]
    wq2 = np.zeros((128, 12, 2, 128), np.float32)
    for t_i, T in enumerate(terms):
        # T: (256 oc, 512 j) -> lhsT [jl, jh, oc] per (och, jstep)
        for och in range(2):
            for jstep in range(2):
                blk = T[128 * och:128 * (och + 1),
                        256 * jstep:256 * (jstep + 1)]  # (128 oc, 256 j)
                wq2[:, t_i * 4 + och * 2 + jstep, :, :] = (
                    blk.T.reshape(2, 128, 128).transpose(1, 0, 2))
    wq2 = q8(wq2)

    in_maps = []
    for core in range(8):
        b, h = core // 2, core % 2
        scols = np.r_[1024 * h:1024 * (h + 1),
                      2048 + 1024 * h:2048 + 1024 * (h + 1)]

        def strip3(x):
            return np.ascontiguousarray(
                x[:, scols].reshape(2, 128, 2048).transpose(1, 0, 2))

        b8q = B8[b].reshape(2, 128, 4, 1024)[:, :, [0, 2, 1, 3], :]
        in_maps.append({
            "B8": np.ascontiguousarray(
                b8q.transpose(1, 0, 2, 3).reshape(128, 2, 2, 2048)),
            "AH8": strip3(A8[b]),
            "BH8": strip3(B8[b]),
            "ALO": strip3(Alo[b]),
            "BLO": strip3(Blo[b]),
            "WQ1": wq1,
            "WQ2": wq2,
        })
    return in_maps


def _combine(results):
    out = np.zeros((4, 256, 4096), dtype=np.float32)
    for core in range(8):
        b, h = core // 2, core % 2
        om = results[core]["OM8"].astype(np.float32).reshape(256, 4096)
        out[b] += om * (2.0 ** -9)
        ow = results[core]["OW"].astype(np.float32).reshape(256, 2048)
        out[b][:, 1024 * h:1024 * (h + 1)] += ow[:, 0:1024]
        out[b][:, 2048 + 1024 * h:2048 + 1024 * (h + 1)] += ow[:, 1024:2048]
    return out.reshape(4, 256, 64, 64)


def run(inputs, **kwargs):
    nc = _get_nc()
    in_maps = _prep_inputs(**inputs)
    try:
        res = run_bass_kernel_spmd(nc, in_maps, core_ids=list(range(8)), **kwargs)
    except Exception:
        # transient NRT device wedge: retry once
        res = run_bass_kernel_spmd(nc, in_maps, core_ids=list(range(8)), **kwargs)
    return _combine(res.results), res


def kernel(A, B, W_phi, W_theta, W_g, W_AB, W_mask):
    out, _ = run(dict(A=A, B=B, W_phi=W_phi, W_theta=W_theta, W_g=W_g,
                      W_AB=W_AB, W_mask=W_mask))
    return out


if __name__ == "__main__":
    rng = np.random.default_rng(0)
    ins = {
        "A": rng.standard_normal((4, 256, 64, 64)).astype(np.float32),
        "B": rng.standard_normal((4, 256, 64, 64)).astype(np.float32),
        "W_phi": (rng.standard_normal((128, 256)) * 0.02).astype(np.float32),
        "W_theta": (rng.standard_normal((128, 256)) * 0.02).astype(np.float32),
        "W_g": (rng.standard_normal((128, 512)) * 0.02).astype(np.float32),
        "W_AB": (rng.standard_normal((256, 512)) * 0.02).astype(np.float32),
        "W_mask": (rng.standard_normal((256, 128)) * 0.02).astype(np.float32),
    }
    out = kernel(**ins)
    print("kernel out", out.shape, out.dtype, float(np.abs(out).max()))


# revision 26
# speedup vs baseline: 1.0058x; 1.0058x over previous
"""Trainium2 Bass kernel for nn_CFI_Module (non-local attention block), fp8.

Per batch b (c=256, h=w=64 -> S=4096 spatial, viewed N=2048):
  phi   = W_phi   @ A   viewed (256, 2048);  theta = W_theta @ B likewise
  g     = W_g     @ [A;B] viewed (256, 2048)
  scores[n, m] = sum_cc theta_v[cc, n] phi_v[cc, m]
  attn = softmax over n (per column m);  y = attn^T-apply;  out = W_mask @ y + W_AB @ [A;B]

Sharding: 8 cores = 4 batches x 2-way split of the m (softmax-free) dim.
Softmax over n is core-local.  Host adds the two per-batch om partials; the
W_AB skip conv is per-strip (disjoint) per core.

Numerics (validated vs fp32 reference in numpy: l2 ~ 3.4e-3):
  - all attention-path matmuls in fp8 with DoubleRow perf mode (2 K-subtiles
    per instruction, 0.5 PE cycles per output element)
  - viewed channel p = 2*ch + hh (conv channel ch, spatial half hh), so the
    256-deep contractions split naturally into hh subtile pairs for DoubleRow
  - weights pre-scaled by 2^6 on host (e4m3-friendly), descaled via the exp
    activation scale (2^-12) and drain-time constant rescales
  - E = exp(scores) stored e5m2 unnormalized (max e^10.9 < e5m2 max)
  - GT = g^T * 2^14/Z stored e4m3 (fits normal range); y accumulates in f32
    PSUM, drained * 2^-7 to e4m3; om output = om * 2^9 in e4m3 (host unscale)
  - W_AB skip conv in fp8 hi/lo decomposition: W@x ~= Wh@xh + (Wh/16)@xl'
    + (Wl/16)@xh with xl' = 16*(x - fp8(x)); output f16 (dominant term,
    ~0.5% error, well within the 2e-2 gate)
"""
import sys

for _p in ("/opt/trn_rl_repo", "/root/.axon_site/_ro/trn_rl_repo"):
    if _p not in sys.path:
        sys.path.append(_p)

import numpy as np
from contextlib import ExitStack

import ml_dtypes
import concourse.bacc as bacc
import concourse.tile as tile
from concourse import mybir
from concourse.bass_utils import run_bass_kernel_spmd

F32 = mybir.dt.float32
F16 = mybir.dt.float16
E4 = mybir.dt.float8e4
E5 = mybir.dt.float8e5
E4NP = ml_dtypes.float8_e4m3
E5NP = ml_dtypes.float8_e5m2
F16NP = np.float16
DR = mybir.MatmulPerfMode.DoubleRow
EXP = mybir.ActivationFunctionType.Exp
MUL = mybir.AluOpType.mult

_NC_CACHE = {}

# resident y blocks (st, nb) accumulated during the exp chain (PSUM-limited)
RESIDENT = [(0, 0), (0, 1), (1, 0)]


def build_nc():
    nc = bacc.Bacc(target_bir_lowering=False, trn_type="TRN2")

    # ---- DRAM I/O ----
    B8_d = nc.dram_tensor("B8", [128, 2, 2, 2048], E4, kind="ExternalInput")
    AH8_d = nc.dram_tensor("AH8", [128, 2, 2048], E4, kind="ExternalInput")
    BH8_d = nc.dram_tensor("BH8", [128, 2, 2048], E4, kind="ExternalInput")
    ALO_d = nc.dram_tensor("ALO", [128, 2, 2048], E4, kind="ExternalInput")
    BLO_d = nc.dram_tensor("BLO", [128, 2, 2048], E4, kind="ExternalInput")
    # wth(2) wph(2) wgA(2) wgB(2) wmk(2)
    WQ1_d = nc.dram_tensor("WQ1", [128, 10, 128], E4, kind="ExternalInput")
    # wab: term(3) x och(2) x jstep(2) -> idx = term*4 + och*2 + jstep
    WQ2_d = nc.dram_tensor("WQ2", [128, 12, 2, 128], E4, kind="ExternalInput")
    OM_d = nc.dram_tensor("OM8", [2, 128, 4096], E4, kind="ExternalOutput")
    OW_d = nc.dram_tensor("OW", [2, 128, 2048], F16, kind="ExternalOutput")

    with tile.TileContext(nc) as tc:
        with ExitStack() as ctx:
            io = ctx.enter_context(tc.tile_pool(name="io", bufs=1))
            acts = ctx.enter_context(tc.tile_pool(name="acts", bufs=1))
            spool = ctx.enter_context(tc.tile_pool(name="spool", bufs=8))
            stg = ctx.enter_context(tc.tile_pool(name="stg", bufs=4))
            wstg = ctx.enter_context(tc.tile_pool(name="wstg", bufs=2))
            psS = ctx.enter_context(tc.tile_pool(name="psS", bufs=2, space="PSUM"))
            psY = ctx.enter_context(tc.tile_pool(name="psY", bufs=2, space="PSUM"))
            psG = ctx.enter_context(tc.tile_pool(name="psG", bufs=1, space="PSUM"))
            psW = ctx.enter_context(tc.tile_pool(name="psW", bufs=1, space="PSUM"))

            # ---- SBUF ----
            b8 = io.tile([128, 2, 2, 2048], E4, name="b8")
            ah8 = io.tile([128, 2, 2048], E4, name="ah8")
            bh8 = io.tile([128, 2, 2048], E4, name="bh8")
            alo = io.tile([128, 2, 2048], E4, name="alo")
            blo = io.tile([128, 2, 2048], E4, name="blo")
            wq1 = io.tile([128, 10, 128], E4, name="wq1")
            wq2 = io.tile([128, 12, 2, 128], E4, name="wq2")
            TH = acts.tile([128, 2, 2048], E4, name="TH")
            PH = acts.tile([128, 2, 1024], E4, name="PH")
            Et = acts.tile([128, 8, 2048], E5, name="Et")
            GT = acts.tile([128, 2, 8, 128], E4, name="GT")
            Y8 = acts.tile([128, 2, 2048], E4, name="Y8")

            wth = wq1[:, 0:2, :]
            wph = wq1[:, 2:4, :]
            wgA = wq1[:, 4:6, :]
            wgB = wq1[:, 6:8, :]

            # ---- input DMAs, ordered by first use ----
            nc.sync.dma_start(out=b8[:, :, 1, 0:1024], in_=B8_d[:, :, 1, 0:1024])
            nc.sync.dma_start(out=wq1, in_=WQ1_d[:, :, :])
            nc.sync.dma_start(out=b8[:, :, 1, 1024:2048],
                              in_=B8_d[:, :, 1, 1024:2048])
            # phi chunk 0 needs strip cols [0:128] and [1024:1152]
            nc.sync.dma_start(out=ah8[:, :, 0:128], in_=AH8_d[:, :, 0:128])
            nc.sync.dma_start(out=ah8[:, :, 1024:1152],
                              in_=AH8_d[:, :, 1024:1152])
            nc.sync.dma_start(out=ah8[:, :, 128:640], in_=AH8_d[:, :, 128:640])
            nc.sync.dma_start(out=ah8[:, :, 1152:1664],
                              in_=AH8_d[:, :, 1152:1664])
            nc.sync.dma_start(out=b8[:, :, 0, :], in_=B8_d[:, :, 0, :])
            nc.sync.dma_start(out=ah8[:, :, 640:1024], in_=AH8_d[:, :, 640:1024])
            nc.sync.dma_start(out=ah8[:, :, 1664:2048],
                              in_=AH8_d[:, :, 1664:2048])
            nc.sync.dma_start(out=bh8, in_=BH8_d[:, :, :])
            nc.sync.dma_start(out=wq2, in_=WQ2_d[:, :, :, :])
            nc.sync.dma_start(out=alo, in_=ALO_d[:, :, :])
            nc.sync.dma_start(out=blo, in_=BLO_d[:, :, :])

            # ---- theta conv: TH[ch, hh, n] over b8 quarters ----
            def theta_q(q, drain="v", pool="s"):
                qp, off = q % 2, 1024 * (q // 2)
                hh, nh = divmod(q, 2)
                dst = TH[:, hh, 1024 * nh:1024 * (nh + 1)]
                if pool == "s":
                    tp = psS.tile([128, 1024], F32, tag="s", name="tp")
                    for jj in range(2):
                        nc.tensor.matmul(
                            tp[:, 512 * jj:512 * (jj + 1)],
                            wth,
                            b8[:, :, qp, off + 512 * jj:off + 512 * (jj + 1)],
                            start=True, stop=True, perf_mode=DR,
                        )
                    if drain == "a":
                        nc.scalar.copy(dst, tp)
                    else:
                        nc.vector.tensor_copy(dst[:, 0:512], tp[:, 0:512])
                        nc.vector.tensor_copy(dst[:, 512:1024], tp[:, 512:1024])
                else:
                    for jj in range(2):
                        tp = psY.tile([128, 512], F32, tag="acc", name="tp")
                        nc.tensor.matmul(
                            tp, wth,
                            b8[:, :, qp, off + 512 * jj:off + 512 * (jj + 1)],
                            start=True, stop=True, perf_mode=DR,
                        )
                        nc.vector.tensor_copy(
                            dst[:, 512 * jj:512 * (jj + 1)], tp)

            theta_q(1)
            theta_q(3, "a")

            # ---- phi conv: PH[ch, hh, m] over strip-col ranges ----
            def phi_range(t0, t1, engine):
                pp = psY.tile([128, 512], F32, tag="acc", name="pp")
                nc.tensor.matmul(pp[:, 0:t1 - t0], wph, ah8[:, :, t0:t1],
                                 start=True, stop=True, perf_mode=DR)
                hh, m0 = divmod(t0, 1024)
                dst = PH[:, hh, m0:m0 + (t1 - t0)]
                if engine == "a":
                    nc.scalar.copy(dst, pp[:, 0:t1 - t0])
                else:
                    nc.vector.tensor_copy(dst, pp[:, 0:t1 - t0])

            phi_range(0, 128, "v")
            phi_range(1024, 1152, "a")
            phi_range(128, 640, "v")
            phi_range(1152, 1664, "v")

            # ---- ow (W_AB skip conv, fp8 hi/lo) job pieces ----
            # cg indexes 512-wide strip-col groups; och the out-channel half.
            ow_stages = {}

            def ow_job(och, cg):
                fw = psW.tile([128, 512], F32, tag="w", name="fw")
                c0, c1 = 512 * cg, 512 * (cg + 1)
                movers = {
                    0: (ah8[:, :, c0:c1], bh8[:, :, c0:c1]),
                    1: (alo[:, :, c0:c1], blo[:, :, c0:c1]),
                    2: (ah8[:, :, c0:c1], bh8[:, :, c0:c1]),
                }
                first = True
                with tc.high_priority(offset=-400000):
                    for term in range(3):
                        for jstep in range(2):
                            nc.tensor.matmul(
                                fw,
                                wq2[:, term * 4 + och * 2 + jstep, :, :],
                                movers[term][jstep],
                                start=first,
                                stop=(term == 2 and jstep == 1),
                                perf_mode=DR,
                            )
                            first = False
                key = (och, cg // 2)
                if key not in ow_stages:
                    ow_stages[key] = wstg.tile([128, 1024], F16, tag="wst",
                                               name=f"ow{och}_{cg // 2}")
                st_t = ow_stages[key]
                nc.vector.tensor_scalar_mul(
                    st_t[:, 512 * (cg % 2):512 * (cg % 2 + 1)], fw, 2.0 ** -6)
                if cg % 2 == 1:
                    nc.sync.dma_start(
                        out=OW_d[och, :, 1024 * (cg // 2):1024 * (cg // 2 + 1)],
                        in_=st_t,
                    )

            ow_jobs = [(och, cg) for cg in range(4) for och in range(2)]

            # ---- scores + exp chain, with GT convs and in-loop y ----
            zs = []

            def scores_exp(k, h2):
                sp = psS.tile([128, 1024], F32, tag="s", name="sp")
                for jj in range(2):
                    nc.tensor.matmul(
                        sp[:, 512 * jj:512 * (jj + 1)],
                        PH[:, :, 128 * k:128 * (k + 1)],
                        TH[:, :, 1024 * h2 + 512 * jj:1024 * h2 + 512 * (jj + 1)],
                        start=True, stop=True, perf_mode=DR,
                    )
                nc.scalar.activation(
                    out=Et[:, k, 1024 * h2:1024 * (h2 + 1)],
                    in_=sp,
                    func=EXP,
                    scale=2.0 ** -12,
                    accum_out=zs[k][:, h2:h2 + 1],
                )

            def gt_k(k, hot=False):
                # Z recip (fused *256 into the drain)
                from contextlib import nullcontext
                hp = tc.high_priority() if hot else nullcontext()
                with hp:
                    nc.vector.tensor_add(zs[k][:, 2:3], zs[k][:, 0:1],
                                         zs[k][:, 1:2])
                    nc.vector.reciprocal(zs[k][:, 3:4], zs[k][:, 2:3])
                gp = psG.tile([128, 512], F32, tag="g", name="gp")
                for st in range(2):
                    with tc.high_priority(offset=-500000):
                        nc.tensor.matmul(
                            gp[:, 128 * st:128 * (st + 1)],
                            ah8[:, :, 1024 * st + 128 * k:1024 * st + 128 * (k + 1)],
                            wgA,
                            start=True, stop=False, perf_mode=DR,
                        )
                        nc.tensor.matmul(
                            gp[:, 128 * st:128 * (st + 1)],
                            bh8[:, :, 1024 * st + 128 * k:
                                1024 * st + 128 * (k + 1)],
                            wgB,
                            start=False, stop=True, perf_mode=DR,
                        )
                hp2 = tc.high_priority() if hot else nullcontext()
                with hp2:
                    nc.vector.tensor_scalar(
                        GT[:, :, k, :], gp[:, 0:256],
                        zs[k][:, 3:4], 256.0, op0=MUL, op1=MUL,
                    )

            y_ps = {}
            y_pool_rr = [psY, psY, psW]

            def y_step(st, nb, p):
                if (st, nb) not in y_ps:
                    pool = y_pool_rr.pop(0) if y_pool_rr else psY
                    y_ps[(st, nb)] = pool.tile([128, 512], F32,
                                               tag="acc" if pool is psY else
                                               ("w" if pool is psW else "g"),
                                               name=f"y{st}{nb}")
                with tc.high_priority(offset=-1000000):
                    nc.tensor.matmul(
                        y_ps[(st, nb)],
                        GT[:, st, 2 * p:2 * p + 2, :],
                        Et[:, 2 * p:2 * p + 2, 512 * nb:512 * (nb + 1)],
                        start=(p == 0), stop=(p == 3),
                        perf_mode=DR,
                    )

            # chain order: two n-half sweeps.  Sweep 1 (h2=0) is gated only
            # by B quarters q0/q2 + phi; sweep 2 (h2=1) by q1/q3 whose theta
            # convs/drains overlap sweep 1.  GT (needs Z = both halves) and
            # in-loop y ride sweep 2.
            for k in range(8):
                zs.append(spool.tile([128, 4], F32, tag="z", name=f"z{k}"))
            ow_i = 0
            for k in range(8):
                scores_exp(k, 1)
                if k == 3:
                    theta_q(0, pool="y")
                    theta_q(2, pool="y")
                    phi_range(640, 1024, "v")
                    phi_range(1664, 2048, "v")
                if k >= 4:
                    for _ in range(2):
                        if ow_i < len(ow_jobs):
                            ow_job(*ow_jobs[ow_i])
                            ow_i += 1
            for k in range(8):
                scores_exp(k, 0)
                gt_k(k, hot=(k >= 6))
                if k % 2 == 1 and k < 7:
                    for (st, nb) in RESIDENT:
                        y_step(st, nb, (k - 1) // 2)

            # ---- tail: finish y, drain Y8, mask conv, om out ----
            def y_drain(st, nb, engine):
                t = y_ps.pop((st, nb))
                dst = Y8[:, st, 512 * nb:512 * (nb + 1)]
                with tc.high_priority():
                    if engine == "v":
                        nc.vector.tensor_scalar_mul(dst, t, 2.0 ** -7)
                    else:
                        nc.scalar.mul(dst, t, 2.0 ** -7)

            om_rr = []
            om_eng = [0]
            om_stage = {}

            def om_single(st, nb, och):
                # psum: rotate over donated pools / psS halves
                kind, pool = om_rr[0]
                om_rr.append(om_rr.pop(0))
                if kind == "half":
                    base, half = pool
                    f = base[:, 512 * half:512 * (half + 1)]
                else:
                    f = pool.tile([128, 512], F32,
                                  tag="acc" if pool is psY else
                                  ("w" if pool is psW else "g"),
                                  name="f_om")
                nc.tensor.matmul(
                    f, wq1[:, 8 + och, :],
                    Y8[:, st, 512 * nb:512 * (nb + 1)],
                    start=True, stop=True,
                )
                key = (st, och)
                if key not in om_stage:
                    om_stage[key] = [stg.tile([128, 2048], E4, tag="om",
                                              name="s_om"), 0]
                ent = om_stage[key]
                sg = ent[0][:, 512 * nb:512 * (nb + 1)]
                # 10:6 ACT:DVE split (ACT is otherwise idle in the tail)
                if om_eng[0] % 8 in (0, 1, 3, 4, 6):
                    nc.scalar.copy(sg, f)
                else:
                    nc.vector.tensor_copy(sg, f)
                om_eng[0] += 1
                ent[1] += 1
                if ent[1] == 4:
                    nc.sync.dma_start(
                        out=OM_d[och, :, 2048 * st:2048 * (st + 1)],
                        in_=om_stage.pop(key)[0],
                    )

            # tail: psG frees after the last GT -> widen the y rotation
            y_pool_rr.extend([psG])
            for (st, nb) in RESIDENT:
                y_step(st, nb, 3)
            # om psum rotation: two psS halves + donated pools as they free up
            fS1 = psS.tile([128, 1024], F32, tag="s", name="fS1")
            fS2 = psS.tile([128, 1024], F32, tag="s", name="fS2")
            om_rr.extend([("half", (fS1, 0)), ("half", (fS1, 1)),
                          ("half", (fS2, 0)), ("half", (fS2, 1))])
            ei = 0
            for (st, nb) in RESIDENT:
                y_drain(st, nb, "a" if ei % 2 == 0 else "v")
                ei += 1
                for och in range(2):
                    om_single(st, nb, och)
            rest = [(st, nb) for st in range(2) for nb in range(4)
                    if (st, nb) not in RESIDENT]
            for (st, nb) in rest:
                for p in range(4):
                    y_step(st, nb, p)
                y_drain(st, nb, "v" if ei % 2 == 0 else "a")
                ei += 1
                for och in range(2):
                    om_single(st, nb, och)

    nc.compile()
    return nc


def _get_nc():
    if "nc" not in _NC_CACHE:
        _NC_CACHE["nc"] = build_nc()
    return _NC_CACHE["nc"]


def _prep_inputs(A, B, W_phi, W_theta, W_g, W_AB, W_mask):
    A = np.ascontiguousarray(np.asarray(A, np.float32)).reshape(4, 256, 4096)
    B = np.ascontiguousarray(np.asarray(B, np.float32)).reshape(4, 256, 4096)
    W_phi = np.asarray(W_phi, np.float32)
    W_theta = np.asarray(W_theta, np.float32)
    W_g = np.asarray(W_g, np.float32)
    W_AB = np.asarray(W_AB, np.float32)
    W_mask = np.asarray(W_mask, np.float32)

    def q8(x):
        return np.clip(x, -240.0, 240.0).astype(E4NP)

    A8 = q8(A)
    B8 = q8(B)
    Alo = q8((A - A8.astype(np.float32)) * 16.0)
    Blo = q8((B - B8.astype(np.float32)) * 16.0)

    def chansplit(w):
        # (ch_out, 256) -> [128 icl, 2 ich, ch_out]
        t = w.T.reshape(2, 128, -1)  # (ich, icl, ch)
        return t.transpose(1, 0, 2)

    wq1 = np.zeros((128, 10, 128), np.float32)
    wq1[:, 0:2, :] = chansplit(W_theta * 64.0)
    wq1[:, 2:4, :] = chansplit(W_phi * 64.0)
    wg = (W_g * 64.0).T.reshape(4, 128, 128).transpose(1, 0, 2)  # [jl, jc, chg]
    wq1[:, 4:6, :] = wg[:, 0:2, :]
    wq1[:, 6:8, :] = wg[:, 2:4, :]
    wmkT = (W_mask * 4.0).T  # (128 i, 256 oc)
    wq1[:, 8, :] = wmkT[:, 0:128]
    wq1[:, 9, :] = wmkT[:, 128:256]
    wq1 = q8(wq1)

    Whi = q8(W_AB * 64.0)
    Wlo = q8((W_AB * 64.0 - Whi.astype(np.float32)) * 16.0)
    terms = [Whi.astype(np.float32), Whi.astype(np.float32) / 16.0,
             Wlo.astype(np.float32) / 16.0]
    wq2 = np.zeros((128, 12, 2, 128), np.float32)
    for t_i, T in enumerate(terms):
        # T: (256 oc, 512 j) -> lhsT [jl, jh, oc] per (och, jstep)
        for och in range(2):
            for jstep in range(2):
                blk = T[128 * och:128 * (och + 1),
                        256 * jstep:256 * (jstep + 1)]  # (128 oc, 256 j)
                wq2[:, t_i * 4 + och * 2 + jstep, :, :] = (
                    blk.T.reshape(2, 128, 128).transpose(1, 0, 2))
    wq2 = q8(wq2)

    in_maps = []
    for core in range(8):
        b, h = core // 2, core % 2
        scols = np.r_[1024 * h:1024 * (h + 1),
                      2048 + 1024 * h:2048 + 1024 * (h + 1)]

        def strip3(x):
            return np.ascontiguousarray(
                x[:, scols].reshape(2, 128, 2048).transpose(1, 0, 2))

        b8q = B8[b].reshape(2, 128, 4, 1024)[:, :, [0, 2, 1, 3], :]
        in_maps.append({
            "B8": np.ascontiguousarray(
                b8q.transpose(1, 0, 2, 3).reshape(128, 2, 2, 2048)),
            "AH8": strip3(A8[b]),
            "BH8": strip3(B8[b]),
            "ALO": strip3(Alo[b]),
            "BLO": strip3(Blo[b]),
            "WQ1": wq1,
            "WQ2": wq2,
        })
    return in_maps


def _combine(results):
    out = np.zeros((4, 256, 4096), dtype=np.float32)
    for core in range(8):
        b, h = core // 2, core % 2
        om = results[core]["OM8"].astype(np.float32).reshape(256, 4096)
        out[b] += om * (2.0 ** -9)
        ow = results[core]["OW"].astype(np.float32).reshape(256, 2048)
        out[b][:, 1024 * h:1024 * (h + 1)] += ow[:, 0:1024]
        out[b][:, 2048 + 1024 * h:2048 + 1024 * (h + 1)] += ow[:, 1024:2048]
    return out.reshape(4, 256, 64, 64)


def run(inputs, **kwargs):
    nc = _get_nc()
    in_maps = _prep_inputs(**inputs)
    try:
        res = run_bass_kernel_spmd(nc, in_maps, core_ids=list(range(8)), **kwargs)
    except Exception:
        # transient NRT device wedge: retry once
        res = run_bass_kernel_spmd(nc, in_maps, core_ids=list(range(8)), **kwargs)
    return _combine(res.results), res


def kernel(A, B, W_phi, W_theta, W_g, W_AB, W_mask):
    out, _ = run(dict(A=A, B=B, W_phi=W_phi, W_theta=W_theta, W_g=W_g,
                      W_AB=W_AB, W_mask=W_mask))
    return out


if __name__ == "__main__":
    rng = np.random.default_rng(0)
    ins = {
        "A": rng.standard_normal((4, 256, 64, 64)).astype(np.float32),
        "B": rng.standard_normal((4, 256, 64, 64)).astype(np.float32),
        "W_phi": (rng.standard_normal((128, 256)) * 0.02).astype(np.float32),
        "W_theta": (rng.standard_normal((128, 256)) * 0.02).astype(np.float32),
        "W_g": (rng.standard_normal((128, 512)) * 0.02).astype(np.float32),
        "W_AB": (rng.standard_normal((256, 512)) * 0.02).astype(np.float32),
        "W_mask": (rng.standard_normal((256, 128)) * 0.02).astype(np.float32),
    }
    out = kernel(**ins)
    print("kernel out", out.shape, out.dtype, float(np.abs(out).max()))


# revision 27
# speedup vs baseline: 1.0281x; 1.0222x over previous
"""Trainium2 Bass kernel for nn_CFI_Module (non-local attention block), fp8.

Per batch b (c=256, h=w=64 -> S=4096 spatial, viewed N=2048):
  phi   = W_phi   @ A   viewed (256, 2048);  theta = W_theta @ B likewise
  g     = W_g     @ [A;B] viewed (256, 2048)
  scores[n, m] = sum_cc theta_v[cc, n] phi_v[cc, m]
  attn = softmax over n (per column m);  y = attn^T-apply;  out = W_mask @ y + W_AB @ [A;B]

Sharding: 8 cores = 4 batches x 2-way split of the m (softmax-free) dim.
Softmax over n is core-local.  Host adds the two per-batch om partials; the
W_AB skip conv is per-strip (disjoint) per core.

Numerics (validated vs fp32 reference in numpy: l2 ~ 3.4e-3):
  - all attention-path matmuls in fp8 with DoubleRow perf mode (2 K-subtiles
    per instruction, 0.5 PE cycles per output element)
  - viewed channel p = 2*ch + hh (conv channel ch, spatial half hh), so the
    256-deep contractions split naturally into hh subtile pairs for DoubleRow
  - weights pre-scaled by 2^6 on host (e4m3-friendly), descaled via the exp
    activation scale (2^-12) and drain-time constant rescales
  - E = exp(scores) stored e5m2 unnormalized (max e^10.9 < e5m2 max)
  - GT = g^T * 2^14/Z stored e4m3 (fits normal range); y accumulates in f32
    PSUM, drained * 2^-7 to e4m3; om output = om * 2^9 in e4m3 (host unscale)
  - W_AB skip conv in fp8 hi/lo decomposition: W@x ~= Wh@xh + (Wh/16)@xl'
    + (Wl/16)@xh with xl' = 16*(x - fp8(x)); output f16 (dominant term,
    ~0.5% error, well within the 2e-2 gate)
"""
import sys

for _p in ("/opt/trn_rl_repo", "/root/.axon_site/_ro/trn_rl_repo"):
    if _p not in sys.path:
        sys.path.append(_p)

import numpy as np
from contextlib import ExitStack

import ml_dtypes
import concourse.bacc as bacc
import concourse.tile as tile
from concourse import mybir
from concourse.bass_utils import run_bass_kernel_spmd

F32 = mybir.dt.float32
F16 = mybir.dt.float16
E4 = mybir.dt.float8e4
E5 = mybir.dt.float8e5
E4NP = ml_dtypes.float8_e4m3
E5NP = ml_dtypes.float8_e5m2
F16NP = np.float16
DR = mybir.MatmulPerfMode.DoubleRow
EXP = mybir.ActivationFunctionType.Exp
MUL = mybir.AluOpType.mult

_NC_CACHE = {}

# resident y blocks (st, nb) accumulated during the exp chain (PSUM-limited)
RESIDENT = [(0, 0), (0, 1), (1, 0)]


def build_nc():
    nc = bacc.Bacc(target_bir_lowering=False, trn_type="TRN2")

    # ---- DRAM I/O ----
    B8_d = nc.dram_tensor("B8", [128, 2, 2, 2048], E4, kind="ExternalInput")
    AH8_d = nc.dram_tensor("AH8", [128, 2, 2048], E4, kind="ExternalInput")
    BH8_d = nc.dram_tensor("BH8", [128, 2, 2048], E4, kind="ExternalInput")
    ALO_d = nc.dram_tensor("ALO", [128, 2, 2048], E4, kind="ExternalInput")
    BLO_d = nc.dram_tensor("BLO", [128, 2, 2048], E4, kind="ExternalInput")
    # wth(2) wph(2) wgA(2) wgB(2) wmk(2)
    WQ1_d = nc.dram_tensor("WQ1", [128, 10, 128], E4, kind="ExternalInput")
    # wab: term(3) x och(2) x jstep(2) -> idx = term*4 + och*2 + jstep
    WQ2_d = nc.dram_tensor("WQ2", [128, 12, 2, 128], E4, kind="ExternalInput")
    OM_d = nc.dram_tensor("OM8", [2, 128, 4096], E4, kind="ExternalOutput")
    OW_d = nc.dram_tensor("OW", [2, 128, 2048], F16, kind="ExternalOutput")

    with tile.TileContext(nc) as tc:
        with ExitStack() as ctx:
            io = ctx.enter_context(tc.tile_pool(name="io", bufs=1))
            acts = ctx.enter_context(tc.tile_pool(name="acts", bufs=1))
            spool = ctx.enter_context(tc.tile_pool(name="spool", bufs=8))
            stg = ctx.enter_context(tc.tile_pool(name="stg", bufs=4))
            wstg = ctx.enter_context(tc.tile_pool(name="wstg", bufs=2))
            psS = ctx.enter_context(tc.tile_pool(name="psS", bufs=2, space="PSUM"))
            psY = ctx.enter_context(tc.tile_pool(name="psY", bufs=2, space="PSUM"))
            psG = ctx.enter_context(tc.tile_pool(name="psG", bufs=1, space="PSUM"))
            psW = ctx.enter_context(tc.tile_pool(name="psW", bufs=1, space="PSUM"))

            # ---- SBUF ----
            b8 = io.tile([128, 2, 2, 2048], E4, name="b8")
            ah8 = io.tile([128, 2, 2048], E4, name="ah8")
            bh8 = io.tile([128, 2, 2048], E4, name="bh8")
            alo = io.tile([128, 2, 2048], E4, name="alo")
            blo = io.tile([128, 2, 2048], E4, name="blo")
            wq1 = io.tile([128, 10, 128], E4, name="wq1")
            wq2 = io.tile([128, 12, 2, 128], E4, name="wq2")
            TH = acts.tile([128, 2, 2048], E4, name="TH")
            PH = acts.tile([128, 2, 1024], E4, name="PH")
            Et = acts.tile([128, 8, 2048], E5, name="Et")
            GT = acts.tile([128, 2, 8, 128], E4, name="GT")
            Y8 = acts.tile([128, 2, 2048], E4, name="Y8")

            wth = wq1[:, 0:2, :]
            wph = wq1[:, 2:4, :]
            wgA = wq1[:, 4:6, :]
            wgB = wq1[:, 6:8, :]

            # ---- input DMAs, ordered by first use ----
            nc.sync.dma_start(out=b8[:, :, 1, 0:1024], in_=B8_d[:, :, 1, 0:1024])
            nc.sync.dma_start(out=wq1, in_=WQ1_d[:, :, :])
            nc.sync.dma_start(out=b8[:, :, 1, 1024:2048],
                              in_=B8_d[:, :, 1, 1024:2048])
            # phi chunk 0 needs strip cols [0:128] and [1024:1152]
            nc.sync.dma_start(out=ah8[:, :, 0:128], in_=AH8_d[:, :, 0:128])
            nc.sync.dma_start(out=ah8[:, :, 1024:1152],
                              in_=AH8_d[:, :, 1024:1152])
            nc.sync.dma_start(out=ah8[:, :, 128:640], in_=AH8_d[:, :, 128:640])
            nc.sync.dma_start(out=ah8[:, :, 1152:1664],
                              in_=AH8_d[:, :, 1152:1664])
            nc.sync.dma_start(out=b8[:, :, 0, :], in_=B8_d[:, :, 0, :])
            nc.sync.dma_start(out=ah8[:, :, 640:1024], in_=AH8_d[:, :, 640:1024])
            nc.sync.dma_start(out=ah8[:, :, 1664:2048],
                              in_=AH8_d[:, :, 1664:2048])
            nc.sync.dma_start(out=bh8, in_=BH8_d[:, :, :])
            nc.sync.dma_start(out=wq2, in_=WQ2_d[:, :, :, :])
            nc.sync.dma_start(out=alo, in_=ALO_d[:, :, :])
            nc.sync.dma_start(out=blo, in_=BLO_d[:, :, :])

            # ---- theta conv: TH[ch, hh, n] over b8 quarters ----
            def theta_q(q, drain="v", pool="s"):
                qp, off = q % 2, 1024 * (q // 2)
                hh, nh = divmod(q, 2)
                dst = TH[:, hh, 1024 * nh:1024 * (nh + 1)]
                if pool == "s":
                    tp = psS.tile([128, 1024], F32, tag="s", name="tp")
                    for jj in range(2):
                        nc.tensor.matmul(
                            tp[:, 512 * jj:512 * (jj + 1)],
                            wth,
                            b8[:, :, qp, off + 512 * jj:off + 512 * (jj + 1)],
                            start=True, stop=True, perf_mode=DR,
                        )
                    if drain == "a":
                        nc.scalar.copy(dst, tp)
                    else:
                        nc.vector.tensor_copy(dst[:, 0:512], tp[:, 0:512])
                        nc.vector.tensor_copy(dst[:, 512:1024], tp[:, 512:1024])
                else:
                    for jj in range(2):
                        tp = psY.tile([128, 512], F32, tag="acc", name="tp")
                        nc.tensor.matmul(
                            tp, wth,
                            b8[:, :, qp, off + 512 * jj:off + 512 * (jj + 1)],
                            start=True, stop=True, perf_mode=DR,
                        )
                        nc.vector.tensor_copy(
                            dst[:, 512 * jj:512 * (jj + 1)], tp)

            theta_q(1)
            theta_q(3, "a")

            # ---- phi conv: PH[ch, hh, m] over strip-col ranges ----
            def phi_range(t0, t1, engine):
                pp = psY.tile([128, 512], F32, tag="acc", name="pp")
                nc.tensor.matmul(pp[:, 0:t1 - t0], wph, ah8[:, :, t0:t1],
                                 start=True, stop=True, perf_mode=DR)
                hh, m0 = divmod(t0, 1024)
                dst = PH[:, hh, m0:m0 + (t1 - t0)]
                if engine == "a":
                    nc.scalar.copy(dst, pp[:, 0:t1 - t0])
                else:
                    nc.vector.tensor_copy(dst, pp[:, 0:t1 - t0])

            phi_range(0, 128, "v")
            phi_range(1024, 1152, "a")
            phi_range(128, 640, "v")
            phi_range(1152, 1664, "v")

            # ---- ow (W_AB skip conv, fp8 hi/lo) job pieces ----
            # cg indexes 512-wide strip-col groups; och the out-channel half.
            ow_stages = {}

            def ow_job(och, cg):
                fw = psW.tile([128, 512], F32, tag="w", name="fw")
                c0, c1 = 512 * cg, 512 * (cg + 1)
                movers = {
                    0: (ah8[:, :, c0:c1], bh8[:, :, c0:c1]),
                    1: (alo[:, :, c0:c1], blo[:, :, c0:c1]),
                    2: (ah8[:, :, c0:c1], bh8[:, :, c0:c1]),
                }
                first = True
                with tc.high_priority(offset=-400000):
                    for term in range(3):
                        for jstep in range(2):
                            nc.tensor.matmul(
                                fw,
                                wq2[:, term * 4 + och * 2 + jstep, :, :],
                                movers[term][jstep],
                                start=first,
                                stop=(term == 2 and jstep == 1),
                                perf_mode=DR,
                            )
                            first = False
                key = (och, cg // 2)
                if key not in ow_stages:
                    ow_stages[key] = wstg.tile([128, 1024], F16, tag="wst",
                                               name=f"ow{och}_{cg // 2}")
                st_t = ow_stages[key]
                nc.vector.tensor_scalar_mul(
                    st_t[:, 512 * (cg % 2):512 * (cg % 2 + 1)], fw, 2.0 ** -6)
                if cg % 2 == 1:
                    nc.sync.dma_start(
                        out=OW_d[och, :, 1024 * (cg // 2):1024 * (cg // 2 + 1)],
                        in_=st_t,
                    )

            ow_jobs = [(och, cg) for cg in range(4) for och in range(2)]

            # ---- scores + exp chain, with GT convs and in-loop y ----
            zs = []

            def scores_exp(k, h2):
                sp = psS.tile([128, 1024], F32, tag="s", name="sp")
                for jj in range(2):
                    nc.tensor.matmul(
                        sp[:, 512 * jj:512 * (jj + 1)],
                        PH[:, :, 128 * k:128 * (k + 1)],
                        TH[:, :, 1024 * h2 + 512 * jj:1024 * h2 + 512 * (jj + 1)],
                        start=True, stop=True, perf_mode=DR,
                    )
                nc.scalar.activation(
                    out=Et[:, k, 1024 * h2:1024 * (h2 + 1)],
                    in_=sp,
                    func=EXP,
                    scale=2.0 ** -12,
                    accum_out=zs[k][:, h2:h2 + 1],
                )

            def gt_k(k, hot=False):
                # Z recip (fused *256 into the drain)
                from contextlib import nullcontext
                hp = tc.high_priority() if hot else nullcontext()
                with hp:
                    nc.vector.tensor_add(zs[k][:, 2:3], zs[k][:, 0:1],
                                         zs[k][:, 1:2])
                    nc.vector.reciprocal(zs[k][:, 3:4], zs[k][:, 2:3])
                gp = psG.tile([128, 512], F32, tag="g", name="gp")
                for st in range(2):
                    with tc.high_priority(offset=-500000):
                        nc.tensor.matmul(
                            gp[:, 128 * st:128 * (st + 1)],
                            ah8[:, :, 1024 * st + 128 * k:1024 * st + 128 * (k + 1)],
                            wgA,
                            start=True, stop=False, perf_mode=DR,
                        )
                        nc.tensor.matmul(
                            gp[:, 128 * st:128 * (st + 1)],
                            bh8[:, :, 1024 * st + 128 * k:
                                1024 * st + 128 * (k + 1)],
                            wgB,
                            start=False, stop=True, perf_mode=DR,
                        )
                hp2 = tc.high_priority() if hot else nullcontext()
                with hp2:
                    nc.vector.tensor_scalar(
                        GT[:, :, k, :], gp[:, 0:256],
                        zs[k][:, 3:4], 256.0, op0=MUL, op1=MUL,
                    )

            y_ps = {}
            y_pool_rr = [psY, psY, psW]

            def y_step(st, nb, p):
                if (st, nb) not in y_ps:
                    pool = y_pool_rr.pop(0) if y_pool_rr else psY
                    y_ps[(st, nb)] = pool.tile([128, 512], F32,
                                               tag="acc" if pool is psY else
                                               ("w" if pool is psW else "g"),
                                               name=f"y{st}{nb}")
                with tc.high_priority(offset=-1000000):
                    nc.tensor.matmul(
                        y_ps[(st, nb)],
                        GT[:, st, 2 * p:2 * p + 2, :],
                        Et[:, 2 * p:2 * p + 2, 512 * nb:512 * (nb + 1)],
                        start=(p == 0), stop=(p == 3),
                        perf_mode=DR,
                    )

            # chain order: two n-half sweeps.  Sweep 1 (h2=0) is gated only
            # by B quarters q0/q2 + phi; sweep 2 (h2=1) by q1/q3 whose theta
            # convs/drains overlap sweep 1.  GT (needs Z = both halves) and
            # in-loop y ride sweep 2.
            for k in range(8):
                zs.append(spool.tile([128, 4], F32, tag="z", name=f"z{k}"))
            ow_i = 0
            for k in range(8):
                scores_exp(k, 1)
                if k == 3:
                    theta_q(0, pool="y")
                    theta_q(2, pool="y")
                    phi_range(640, 1024, "v")
                    phi_range(1664, 2048, "v")
                if k >= 4:
                    for _ in range(2):
                        if ow_i < len(ow_jobs):
                            ow_job(*ow_jobs[ow_i])
                            ow_i += 1
            for k in range(8):
                scores_exp(k, 0)
                gt_k(k, hot=(k >= 6))
                if k % 2 == 1 and k < 7:
                    for (st, nb) in RESIDENT:
                        y_step(st, nb, (k - 1) // 2)

            # ---- tail: finish y, drain Y8, mask conv, om out ----
            def y_drain(st, nb, engine):
                t = y_ps.pop((st, nb))
                dst = Y8[:, st, 512 * nb:512 * (nb + 1)]
                if engine == "v":
                    nc.vector.tensor_scalar_mul(dst, t, 2.0 ** -7)
                else:
                    nc.scalar.mul(dst, t, 2.0 ** -7)

            om_rr = []
            om_eng = [0]
            om_stage = {}

            def om_single(st, nb, och):
                # psum: rotate over donated pools / psS halves
                kind, pool = om_rr[0]
                om_rr.append(om_rr.pop(0))
                if kind == "half":
                    base, half = pool
                    f = base[:, 512 * half:512 * (half + 1)]
                else:
                    f = pool.tile([128, 512], F32,
                                  tag="acc" if pool is psY else
                                  ("w" if pool is psW else "g"),
                                  name="f_om")
                nc.tensor.matmul(
                    f, wq1[:, 8 + och, :],
                    Y8[:, st, 512 * nb:512 * (nb + 1)],
                    start=True, stop=True,
                )
                key = (st, och)
                if key not in om_stage:
                    om_stage[key] = [stg.tile([128, 2048], E4, tag="om",
                                              name="s_om"), 0]
                ent = om_stage[key]
                sg = ent[0][:, 512 * nb:512 * (nb + 1)]
                # 10:6 ACT:DVE split (ACT is otherwise idle in the tail)
                if om_eng[0] % 8 in (0, 1, 3, 4, 6):
                    nc.scalar.copy(sg, f)
                else:
                    nc.vector.tensor_copy(sg, f)
                om_eng[0] += 1
                ent[1] += 1
                if ent[1] == 4:
                    nc.sync.dma_start(
                        out=OM_d[och, :, 2048 * st:2048 * (st + 1)],
                        in_=om_stage.pop(key)[0],
                    )

            # tail: psG frees after the last GT -> widen the y rotation
            y_pool_rr.extend([psG])
            for (st, nb) in RESIDENT:
                y_step(st, nb, 3)
            # om psum rotation: two psS halves + donated pools as they free up
            fS1 = psS.tile([128, 1024], F32, tag="s", name="fS1")
            fS2 = psS.tile([128, 1024], F32, tag="s", name="fS2")
            om_rr.extend([("half", (fS1, 0)), ("half", (fS1, 1)),
                          ("half", (fS2, 0)), ("half", (fS2, 1))])
            ei = 0
            for (st, nb) in RESIDENT:
                y_drain(st, nb, "a" if ei % 2 == 0 else "v")
                ei += 1
                for och in range(2):
                    om_single(st, nb, och)
            rest = [(st, nb) for st in range(2) for nb in range(4)
                    if (st, nb) not in RESIDENT]
            for (st, nb) in rest:
                for p in range(4):
                    y_step(st, nb, p)
                y_drain(st, nb, "v" if ei % 2 == 0 else "a")
                ei += 1
                for och in range(2):
                    om_single(st, nb, och)

    nc.compile()
    return nc


def _get_nc():
    if "nc" not in _NC_CACHE:
        _NC_CACHE["nc"] = build_nc()
    return _NC_CACHE["nc"]


def _prep_inputs(A, B, W_phi, W_theta, W_g, W_AB, W_mask):
    A = np.ascontiguousarray(np.asarray(A, np.float32)).reshape(4, 256, 4096)
    B = np.ascontiguousarray(np.asarray(B, np.float32)).reshape(4, 256, 4096)
    W_phi = np.asarray(W_phi, np.float32)
    W_theta = np.asarray(W_theta, np.float32)
    W_g = np.asarray(W_g, np.float32)
    W_AB = np.asarray(W_AB, np.float32)
    W_mask = np.asarray(W_mask, np.float32)

    def q8(x):
        return np.clip(x, -240.0, 240.0).astype(E4NP)

    A8 = q8(A)
    B8 = q8(B)
    Alo = q8((A - A8.astype(np.float32)) * 16.0)
    Blo = q8((B - B8.astype(np.float32)) * 16.0)

    def chansplit(w):
        # (ch_out, 256) -> [128 icl, 2 ich, ch_out]
        t = w.T.reshape(2, 128, -1)  # (ich, icl, ch)
        return t.transpose(1, 0, 2)

    wq1 = np.zeros((128, 10, 128), np.float32)
    wq1[:, 0:2, :] = chansplit(W_theta * 64.0)
    wq1[:, 2:4, :] = chansplit(W_phi * 64.0)
    wg = (W_g * 64.0).T.reshape(4, 128, 128).transpose(1, 0, 2)  # [jl, jc, chg]
    wq1[:, 4:6, :] = wg[:, 0:2, :]
    wq1[:, 6:8, :] = wg[:, 2:4, :]
    wmkT = (W_mask * 4.0).T  # (128 i, 256 oc)
    wq1[:, 8, :] = wmkT[:, 0:128]
    wq1[:, 9, :] = wmkT[:, 128:256]
    wq1 = q8(wq1)

    Whi = q8(W_AB * 64.0)
    Wlo = q8((W_AB * 64.0 - Whi.astype(np.float32)) * 16.0)
    terms = [Whi.astype(np.float32), Whi.astype(np.float32) / 16.0,
             Wlo.astype(np.float32) / 16.0]
    wq2 = np.zeros((128, 12, 2, 128), np.float32)
    for t_i, T in enumerate(terms):
        # T: (256 oc, 512 j) -> lhsT [jl, jh, oc] per (och, jstep)
        for och in range(2):
            for jstep in range(2):
                blk = T[128 * och:128 * (och + 1),
                        256 * jstep:256 * (jstep + 1)]  # (128 oc, 256 j)
                wq2[:, t_i * 4 + och * 2 + jstep, :, :] = (
                    blk.T.reshape(2, 128, 128).transpose(1, 0, 2))
    wq2 = q8(wq2)

    in_maps = []
    for core in range(8):
        b, h = core // 2, core % 2
        scols = np.r_[1024 * h:1024 * (h + 1),
                      2048 + 1024 * h:2048 + 1024 * (h + 1)]

        def strip3(x):
            return np.ascontiguousarray(
                x[:, scols].reshape(2, 128, 2048).transpose(1, 0, 2))

        b8q = B8[b].reshape(2, 128, 4, 1024)[:, :, [0, 2, 1, 3], :]
        in_maps.append({
            "B8": np.ascontiguousarray(
                b8q.transpose(1, 0, 2, 3).reshape(128, 2, 2, 2048)),
            "AH8": strip3(A8[b]),
            "BH8": strip3(B8[b]),
            "ALO": strip3(Alo[b]),
            "BLO": strip3(Blo[b]),
            "WQ1": wq1,
            "WQ2": wq2,
        })
    return in_maps


def _combine(results):
    out = np.zeros((4, 256, 4096), dtype=np.float32)
    for core in range(8):
        b, h = core // 2, core % 2
        om = results[core]["OM8"].astype(np.float32).reshape(256, 4096)
        out[b] += om * (2.0 ** -9)
        ow = results[core]["OW"].astype(np.float32).reshape(256, 2048)
        out[b][:, 1024 * h:1024 * (h + 1)] += ow[:, 0:1024]
        out[b][:, 2048 + 1024 * h:2048 + 1024 * (h + 1)] += ow[:, 1024:2048]
    return out.reshape(4, 256, 64, 64)


def run(inputs, **kwargs):
    nc = _get_nc()
    in_maps = _prep_inputs(**inputs)
    try:
        res = run_bass_kernel_spmd(nc, in_maps, core_ids=list(range(8)), **kwargs)
    except Exception:
        # transient NRT device wedge: retry once
        res = run_bass_kernel_spmd(nc, in_maps, core_ids=list(range(8)), **kwargs)
    return _combine(res.results), res


def kernel(A, B, W_phi, W_theta, W_g, W_AB, W_mask):
    out, _ = run(dict(A=A, B=B, W_phi=W_phi, W_theta=W_theta, W_g=W_g,
                      W_AB=W_AB, W_mask=W_mask))
    return out


if __name__ == "__main__":
    rng = np.random.default_rng(0)
    ins = {
        "A": rng.standard_normal((4, 256, 64, 64)).astype(np.float32),
        "B": rng.standard_normal((4, 256, 64, 64)).astype(np.float32),
        "W_phi": (rng.standard_normal((128, 256)) * 0.02).astype(np.float32),
        "W_theta": (rng.standard_normal((128, 256)) * 0.02).astype(np.float32),
        "W_g": (rng.standard_normal((128, 512)) * 0.02).astype(np.float32),
        "W_AB": (rng.standard_normal((256, 512)) * 0.02).astype(np.float32),
        "W_mask": (rng.standard_normal((256, 128)) * 0.02).astype(np.float32),
    }
    out = kernel(**ins)
    print("kernel out", out.shape, out.dtype, float(np.abs(out).max()))
